# revision 3
# baseline (speedup 1.0000x reference)
"""DLRM dot-interaction kernel v14 for Trainium2 (8 NeuronCores, batch-sharded).

Per sample b: T = concat(dense[b], embs[b]) -> [27, 128]; Z = T @ T^T;
output = strict upper triangle of Z -> [351] fp32.

Per-core plan (2048 samples, 16 blocks of 128):
  - Per-block SWDGE fp32->fp16 cast loads (fine dependency granularity).
  - PE feature-slab transposes -> f-major Tt [128 d, f*128+b] fp16;
    PSUM->SBUF copies alternate DVE/ACT.
  - Gram: per-sample col-group-tiled matmuls (sample s = 32g+q), 32x32
    tiles, PSUM zp [128, 512] per 16 samples; Z copies -> Zs fp16
    [(g,m) part, (q, t, n)] with t = block index within group.
  - Backend per group (sizes 8,6,2): scatter Zs -> DRAM scratch packed
    sample-major (4 HWDGE DMAs, 512B runs), readback [s part, (m,t,n)]
    (13.8KB runs), triu-pack to Pk [s, (t,351)] fp32, output DMA (1404B
    runs). All backend work is emitted interleaved into the NEXT
    group's block loop so its DMA waits and copies hide under compute;
    group sizes taper so the final tail is short.
"""

import numpy as np

B, NUM_EMBS, D = 16384, 26, 128
N_CORES = 8
BC = B // N_CORES  # 2048 samples per core
BLK = 128          # samples per block
NF = NUM_EMBS + 1  # 27 features
FP = 32            # feature pitch (27 + 5 pad)
NPAIR = NF * (NF - 1) // 2  # 351

_CACHE = {}


def _group_sizes(nblk):
    if nblk >= 16:
        sizes = []
        rem = nblk
        while rem > 16:
            sizes.append(8)
            rem -= 8
        sizes += [8, 5, 2, 1]
        return sizes
    return [nblk]


def build(bc=BC):
    import concourse.bacc as bacc
    import concourse.mybir as mybir
    from concourse.tile import TileContext
    from concourse.masks import make_identity

    fp16 = mybir.dt.float16
    fp32 = mybir.dt.float32

    nc = bacc.Bacc("TRN2", target_bir_lowering=False, debug=False)
    dense_t = nc.dram_tensor("dense", (bc, D), fp32, kind="ExternalInput")
    embs_t = nc.dram_tensor("embs", (bc, NUM_EMBS, D), fp32, kind="ExternalInput")
    out_t = nc.dram_tensor("out", (bc, NPAIR), fp32, kind="ExternalOutput")

    nblk = bc // BLK
    sizes = _group_sizes(nblk)
    assert sum(sizes) == nblk

    with TileContext(nc) as tc:
        with (
            tc.tile_pool(name="consts", bufs=1) as consts,
            tc.tile_pool(name="xin", bufs=4) as xpool,
            tc.tile_pool(name="dn", bufs=2) as dnpool,
            tc.tile_pool(name="tt", bufs=4) as ttpool,
            tc.tile_pool(name="zsb", bufs=2) as zspool,
            tc.tile_pool(name="zgb", bufs=2) as zgpool,
            tc.tile_pool(name="pk", bufs=2) as pkpool,
            tc.tile_pool(name="tp", bufs=4, space="PSUM") as tppool,
            tc.tile_pool(name="zp", bufs=1, space="PSUM") as zppool,
            tc.tile_pool(name="dscr", bufs=2, space="DRAM") as dpool,
        ):
            dview = dense_t.ap()  # [bc, 128]
            eview = embs_t.ap().rearrange("b f d -> b (f d)")  # [bc, 3328]
            oview = out_t.ap()  # [bc, 351]

            state = {"flip": 0}

            def load_block(blk, chunked=False):
                b0 = blk * BLK
                if not chunked:
                    # embs only; dense comes from the per-group batch load
                    X = xpool.tile([BLK, NUM_EMBS * D], fp16, tag="Xe")
                    nc.gpsimd.dma_start(out=X[:, :], in_=eview[b0 : b0 + BLK])
                    return X
                X = xpool.tile([BLK, NF * D], fp16, tag="X")
                # split by transpose chunk (8 features each) so the first
                # transposes can start after ~1/4 of the block has landed
                nc.gpsimd.dma_start(out=X[:, 0:D], in_=dview[b0 : b0 + BLK])
                for ci in range(4):
                    flo = max(0, 8 * ci - 1)  # embs feature = f - 1
                    fhi = min(NUM_EMBS, 8 * ci + 7)
                    nc.gpsimd.dma_start(
                        out=X[:, (flo + 1) * D : (fhi + 1) * D],
                        in_=eview[b0 : b0 + BLK, flo * D : fhi * D],
                    )
                return X

            # prefetch the first blocks before the identity constant is
            # built so the SWDGE transfers start immediately
            ident = consts.tile([128, 128], fp16)
            X0 = load_block(0, chunked=True)
            # identity build (gpsimd) between the first two block loads:
            # first transfers start immediately, ident ready before the
            # first transposes need it
            make_identity(nc, ident)
            X1 = load_block(1, chunked=True)
            prefetched = {0: X0, 1: X1}

            # Persistent PSUM tiles for Gram output, zero-initialized once.
            # Matmuls then only write the 27x27 live region; pad rows/cols
            # (27..31 of each 32-slot) stay zero forever, so the full
            # [128, 512] Z-copies read defined data and LDW/MM shrink to
            # 27 columns each.
            zp_big = zppool.tile([128, 64 * FP], fp32, tag="zp", name="zp_big")
            zp_tiles = [zp_big[:, 512 * i : 512 * (i + 1)] for i in range(4)]
            for zpt in zp_tiles:
                nc.vector.memset(zpt, 0.0)
            state_zp = {"i": 0}

            def transpose_block(X, dslab):
                # dslab = [128, 128] dense feature slab (f=0); X holds the
                # embs slabs, at f*D (prefetched blocks) or (f-1)*D
                full = dslab is None
                Tt = ttpool.tile([128, FP * D], fp16, tag="Tt")
                nchunk = (NF + 7) // 8
                for ci in range(nchunk):
                    c0 = ci * 8
                    cf = min(8, NF - c0)
                    tp = tppool.tile([128, 8 * BLK], fp16, tag="tp")
                    for j in range(cf):
                        f = c0 + j
                        if f == 0:
                            src_slab = X[:, 0:D] if full else dslab
                        elif full:
                            src_slab = X[:, f * D : (f + 1) * D]
                        else:
                            src_slab = X[:, (f - 1) * D : f * D]
                        nc.tensor.transpose(
                            tp[:, j * BLK : (j + 1) * BLK],
                            src_slab,
                            ident,
                        )
                    dst = Tt[:, c0 * BLK : (c0 + cf) * BLK]
                    src = tp[:, : cf * BLK]
                    if state["flip"] % 2 == 0:
                        nc.vector.tensor_copy(out=dst, in_=src)
                    else:
                        nc.scalar.copy(dst, src)
                    state["flip"] += 1
                return Tt

            def gram_block(Tt, zs4, t):
                Ttr = Tt.rearrange("d (f b) -> d b f", b=BLK)
                for qh in range(2):
                    zp = zp_tiles[state_zp["i"] % 4]
                    state_zp["i"] += 1
                    for qi in range(16):
                        q = qh * 16 + qi
                        for g in range(4):
                            s = 32 * g + q
                            nc.tensor.matmul(
                                zp[32 * g : 32 * g + NF, qi * FP : qi * FP + NF],
                                Ttr[:, s, :NF],
                                Ttr[:, s, :NF],
                                start=True,
                                stop=True,
                                tile_position=(0, 32 * g),
                            )
                    src = zp[:, :].rearrange("p (q n) -> p q n", n=FP)
                    dst = zs4[:, qh * 16 : (qh + 1) * 16, t, :]
                    if state["flip"] % 2 == 0:
                        nc.vector.tensor_copy(out=dst, in_=src)
                    else:
                        nc.scalar.copy(dst, src)
                    state["flip"] += 1

            class Backend:
                """Emits one group's scatter/readback/pack/out in stages so
                the caller can interleave them into the next group's
                block loop (engine streams are in-order; a blocked DMA
                wait must not sit in front of the next group's work)."""

                def __init__(self, grp_b0, tb, Zs_t):
                    self.b0 = grp_b0  # first sample row of this group
                    self.tb = tb      # blocks in this group
                    self.Zs_t = Zs_t
                    self.c = tb * FP
                    self.scr = dpool.tile([128, NF * self.c], fp16, tag="scr")
                    self.Zg = None
                    self.Pk = None
                    self.pack_m = 0
                    self.pack_off = 0

                def scatter(self):
                    sv = self.scr.rearrange(
                        "(g q) (m c) -> g q m c", g=4, m=NF
                    )
                    for g in range(4):
                        in3 = self.Zs_t[32 * g : 32 * g + NF].rearrange(
                            "m (q c) -> m q c", q=32
                        )
                        out3 = sv[g].transpose([1, 0, 2])  # [27 m, 32 q, c]
                        eng = nc.sync if g % 2 == 0 else nc.scalar
                        eng.dma_start(out=out3, in_=in3)

                def readback(self):
                    self.Zg = zgpool.tile([128, NF * self.c], fp16, tag="Zg")
                    nc.sync.dma_start(out=self.Zg[:, :], in_=self.scr[:, :])

                def pack_some(self, nm):
                    if self.Zg is None:
                        self.readback()
                    if self.Pk is None:
                        self.Pk = pkpool.tile([128, self.tb * NPAIR], fp32, tag="Pk")
                    zg4 = self.Zg.rearrange(
                        "p (m t n) -> p m t n", m=NF, t=self.tb
                    )
                    pk3 = self.Pk.rearrange("p (t c) -> p t c", t=self.tb)
                    hi = min(self.pack_m + nm, NF - 1)
                    while self.pack_m < hi:
                        m = self.pack_m
                        ln = NF - 1 - m
                        src = zg4[:, m, :, m + 1 : NF]
                        dst = pk3[:, :, self.pack_off : self.pack_off + ln]
                        if state["flip"] % 2 == 0:
                            nc.vector.tensor_copy(out=dst, in_=src)
                        else:
                            nc.scalar.copy(dst, src)
                        state["flip"] += 1
                        self.pack_off += ln
                        self.pack_m += 1

                def finish(self):
                    self.pack_some(NF)
                    pk3 = self.Pk.rearrange("p (t c) -> p t c", t=self.tb)
                    ov3 = oview[self.b0 : self.b0 + self.tb * BLK].rearrange(
                        "(t p) c -> p t c", p=BLK
                    )
                    nc.sync.dma_start(out=ov3, in_=pk3)

                def step(self, i, total):
                    # stage schedule across the next group's `total` blocks
                    if i == 0:
                        self.readback()
                        return
                    if i == 1 and total > 3:
                        return  # let the readback transfer finish first
                    # spread the 26 pack copies over remaining blocks
                    rem_blocks = total - i
                    rem_packs = (NF - 1) - self.pack_m
                    nm = -(-rem_packs // max(1, rem_blocks))
                    self.pack_some(nm)
                    if i == total - 1:
                        self.finish()

            backlog = None
            blk = 0
            for tb in sizes:
                Zs_t = zspool.tile([128, 32 * tb * FP], fp16, tag="Zs")
                zs4 = Zs_t.rearrange("p (q t n) -> p q t n", q=32, t=tb, n=FP)
                gb0 = blk * BLK
                npre = sum(1 for b in (blk + i for i in range(tb)) if b in prefetched)
                Dn = None
                if npre < tb:
                    # one dense DMA for the group's non-prefetched blocks
                    nd = tb - npre
                    Dn = dnpool.tile([BLK, nd * D], fp16, tag="Dn")
                    dsrc = dview[(blk + npre) * BLK : (blk + tb) * BLK].rearrange(
                        "(t b) d -> b t d", t=nd
                    )
                    nc.gpsimd.dma_start(
                        out=Dn.rearrange("b (t d) -> b t d", t=nd), in_=dsrc
                    )
                for t in range(tb):
                    pre = prefetched.pop(blk, None)
                    X = pre if pre is not None else load_block(blk)
                    dslab = None
                    if pre is None:
                        ti = t - npre
                        dslab = Dn[:, ti * D : (ti + 1) * D]
                    Tt = transpose_block(X, dslab)
                    gram_block(Tt, zs4, t)
                    if backlog is not None:
                        backlog.step(t, tb)
                    blk += 1
                if backlog is not None and backlog.pack_m < NF - 1:
                    backlog.finish()
                backlog = Backend((blk - tb) * BLK, tb, Zs_t)
                backlog.scatter()
            backlog.readback()
            backlog.finish()

    nc.compile()
    return nc


def _get(bc=BC):
    if bc not in _CACHE:
        _CACHE[bc] = build(bc)
    return _CACHE[bc]


def kernel(dense: np.ndarray, embs: np.ndarray) -> np.ndarray:
    from concourse import bass_utils

    dense = np.ascontiguousarray(np.asarray(dense, dtype=np.float32))
    embs = np.ascontiguousarray(np.asarray(embs, dtype=np.float32))
    assert dense.shape == (B, D) and embs.shape == (B, NUM_EMBS, D)

    nc = _get()
    dsh = dense.reshape(N_CORES, BC, D)
    esh = embs.reshape(N_CORES, BC, NUM_EMBS, D)
    in_maps = [{"dense": dsh[i], "embs": esh[i]} for i in range(N_CORES)]
    res = bass_utils.run_bass_kernel_spmd(nc, in_maps, core_ids=list(range(N_CORES)))
    return np.concatenate([r["out"] for r in res.results], axis=0)
